# revision 4
# baseline (speedup 1.0000x reference)
"""Trainium2 Bass kernel for nn_Cropper: 100 bilinear 100x100 crops per image,
8 images data-parallel across 8 NeuronCores.

Device algorithm per core (one image [C,H,W] f16 in DRAM, 100 boxes):
  - dma_gather, descriptor per (box, c, i): the 2-row span rows (y0_i, y0_i+1)
    restricted to the box's 128-aligned x-window (ELEM=1664 contiguous f16:
    row y0 tail + row y0+1 head).  Partition = output row i.
  - DVE vertical blend with per-partition scalar wy_i:
      V32[i, c, x'] = R0 + wy*(R1 - R0)   (f32 out)
  - ap_gather (d=1, f32): per (box, c, u, j) tap V32[i, c, xrel_j + u]
      -> Hh[i, (c, u, j)]
  - DVE horizontal blend with per-free-element wx_j (broadcast to all
    partitions once per call via PE ones-matmul into W_all):
      o[i, c, j] = H0 + wx_j*(H1 - H0)    (f16 out)
  - One HWDGE DMA per 4-box group writes o -> out[m, c, i, j].

Host does only: f16 cast of the image, index/weight packing (vectorized
numpy), f16->f32 upcast of the result.  Runner keeps one jitted executable
per device, donates the previous call's output buffer (no zero upload),
and pipelines H2D/exec/D2H across the 8 cores.
"""
import numpy as np
from contextlib import ExitStack

B, NBOX, C, H, W = 8, 100, 3, 1024, 1024
S = 100
STEP = 128            # dma_gather elem_step (f16 elems; 256B)
ELEM = W + 640        # 2-row span: row-y0 tail + row-y0+1 head (f16 elems)
XW = 640              # x-window elems (128-aligned start, covers w0<=462)
NPAD = 2048
GB = 2                # boxes per dma_gather instruction
AB = 4                # boxes per ap_gather instruction
NGIDX = GB * 3 * 128  # dma_gather descriptors per group
NAIDX = AB * 3 * 2 * S  # ap_gather indices per group
WCH = 20              # wx broadcast chunks of 512

_CACHE = {}


def _box_geometry(boxes_b):
    fb = boxes_b.astype(np.float32)
    x0 = np.floor(fb[:, 0] * np.float32(W))
    y0 = np.floor(fb[:, 1] * np.float32(H))
    w0 = np.maximum(np.floor(fb[:, 2] * np.float32(W)), np.float32(1.0))
    h0 = np.maximum(np.floor(fb[:, 3] * np.float32(H)), np.float32(1.0))
    grid = (np.arange(S, dtype=np.float32) + np.float32(0.5)) / np.float32(S)
    sy = np.clip(grid[None, :] * h0[:, None] - np.float32(0.5),
                 np.float32(0.0), (h0 - np.float32(1.0))[:, None]) + y0[:, None]
    sx = np.clip(grid[None, :] * w0[:, None] - np.float32(0.5),
                 np.float32(0.0), (w0 - np.float32(1.0))[:, None]) + x0[:, None]
    yf = np.floor(sy)
    xf = np.floor(sx)
    wy = (sy - yf).astype(np.float32)
    wx = (sx - xf).astype(np.float32)
    y0i = np.clip(yf, 0, H - 1).astype(np.int64)
    x0i = np.clip(xf, 0, W - 1).astype(np.int64)
    return wy, wx, y0i, x0i


def _wrap16(vals_2d):
    """[nblk, n] -> [16, nblk*(n//16)]; idx i at [i%16, i//16] per block."""
    nb, n = vals_2d.shape
    sw = n // 16
    w = np.zeros((nb, 16, sw), dtype=np.int16)
    idx = np.arange(n)
    w[:, idx % 16, idx // 16] = vals_2d
    return np.ascontiguousarray(w.transpose(1, 0, 2).reshape(16, nb * sw))


def _prep_core(image_b, boxes_b):
    """image_b [C,H,W] f32, boxes_b [NBOX,4] f32 -> device input dict."""
    wy, wx, y0i, x0i = _box_geometry(boxes_b)

    xb = (x0i.min(axis=1) // STEP) * STEP            # [NBOX]
    assert (x0i.max(axis=1) + 1 - xb).max() < XW

    # dma_gather: desc n (within GB group) = (b*3 + c)*128 + p ; p = i
    gv = ((np.arange(C, dtype=np.int64) * (H * W // STEP))[None, :, None]
          + y0i[:, None, :] * (W // STEP)
          + (xb // STEP)[:, None, None])             # [NBOX, C, S]
    assert gv.max() < 32768
    g_full = np.zeros((NBOX, C, 128), dtype=np.int64)
    g_full[:, :, :S] = gv
    gidx16 = _wrap16(g_full.reshape(NBOX // GB, NGIDX).astype(np.int16))

    # ap_gather: idx n (within AB group) = ((b*3 + c)*2 + u)*S + j
    #   value = b*(C*XW) + c*XW + xrel_j + u
    xrel = (x0i - xb[:, None]).reshape(NBOX // AB, AB, 1, 1, S)
    av = (xrel
          + (np.arange(AB, dtype=np.int64) * (C * XW))[None, :, None, None, None]
          + (np.arange(C, dtype=np.int64) * XW)[None, None, :, None, None]
          + np.arange(2, dtype=np.int64)[None, None, None, :, None])
    assert av.max() < AB * C * XW
    agidx16 = _wrap16(av.reshape(NBOX // AB, NAIDX).astype(np.int16))

    wyT = np.zeros((128, NBOX), dtype=np.float32)
    wyT[:S] = wy.T

    wx_all = np.zeros((WCH * 512,), dtype=np.float32)
    wx_all[:NBOX * S] = wx.reshape(-1)
    # 3 rows of 7 chunks (landing on SBUF partitions 0/32/64 for PE bcast)
    wx_rows = np.zeros((3, 7 * 512), dtype=np.float32)
    wx_rows.reshape(-1)[:WCH * 512] = wx_all

    img = np.empty((1, C * H * W + NPAD), dtype=np.float16)
    img[0, :C * H * W] = image_b.reshape(-1)
    img[0, C * H * W:] = 0.0

    return {
        "img": img,
        "gidx": gidx16,
        "agidx": agidx16,
        "wyT": wyT,
        "wxa": wx_rows,
    }


def _build_program(reps=1):
    import concourse.bass as bass
    import concourse.tile as tile
    from concourse import bacc, mybir

    f16 = mybir.dt.float16
    f32 = mybir.dt.float32
    i16 = mybir.dt.int16
    Alu = mybir.AluOpType

    NG = NBOX // GB       # dma_gather groups
    NA = NBOX // AB       # ap_gather groups
    GW = NGIDX // 16      # wrapped gidx cols per group
    AW = NAIDX // 16      # wrapped agidx cols per group

    nc = bacc.Bacc("TRN2", target_bir_lowering=False, debug=False,
                   enable_asserts=False, num_devices=8)
    img_d = nc.dram_tensor("img", [1, C * H * W + NPAD], f16,
                           kind="ExternalInput")
    gidx_d = nc.dram_tensor("gidx", [16, NG * GW], i16, kind="ExternalInput")
    agidx_d = nc.dram_tensor("agidx", [16, NA * AW], i16,
                             kind="ExternalInput")
    wyT_d = nc.dram_tensor("wyT", [128, NBOX], f32, kind="ExternalInput")
    wxa_d = nc.dram_tensor("wxa", [3, 7 * 512], f32, kind="ExternalInput")
    out_d = nc.dram_tensor("out", [NBOX, C, S, S], f16, kind="ExternalOutput")

    with tile.TileContext(nc) as tc, ExitStack() as ctx:
        const = ctx.enter_context(tc.tile_pool(name="const", bufs=1))
        gidx_s = const.tile([128, NG * GW], i16)
        nc.sync.dma_start(gidx_s[0:16], gidx_d.ap())
        nc.sync.dma_start(gidx_s[16:32], gidx_s[0:16])
        nc.sync.dma_start(gidx_s[32:64], gidx_s[0:32])
        nc.sync.dma_start(gidx_s[64:128], gidx_s[0:64])
        agidx_s = const.tile([128, NA * AW], i16)
        nc.sync.dma_start(agidx_s[0:16], agidx_d.ap())
        nc.sync.dma_start(agidx_s[16:32], agidx_s[0:16])
        nc.sync.dma_start(agidx_s[32:64], agidx_s[0:32])
        nc.sync.dma_start(agidx_s[64:128], agidx_s[0:64])
        wyT_s = const.tile([128, NBOX], f32)
        nc.sync.dma_start(wyT_s[:], wyT_d.ap())
        wxa_s = const.tile([65, 7 * 512], f32)
        nc.sync.dma_start(wxa_s[0:1], wxa_d.ap()[0:1])
        nc.sync.dma_start(wxa_s[32:33], wxa_d.ap()[1:2])
        nc.sync.dma_start(wxa_s[64:65], wxa_d.ap()[2:3])
        ones_s = const.tile([65, 128], f32)
        nc.vector.memset(ones_s[:], 1.0)

        # W_all[p, m*S + j] = wx[m, j] broadcast to all partitions via PE
        W_all = const.tile([128, WCH * 512], f32)
        wpool = ctx.enter_context(tc.tile_pool(name="wps", bufs=2,
                                               space="PSUM"))
        for k in range(WCH):
            base, col = 32 * (k // 7), k % 7
            psW = wpool.tile([128, 512], f32, tag="psW")
            nc.tensor.matmul(out=psW[:], lhsT=ones_s[base:base + 1],
                             rhs=wxa_s[base:base + 1,
                                       col * 512:(col + 1) * 512],
                             start=True, stop=True)
            nc.vector.tensor_copy(out=W_all[:, k * 512:(k + 1) * 512],
                                  in_=psW[:])

        nrow = (C * H * W + NPAD - ELEM) // STEP
        in_view = bass.AP(img_d.ap().tensor, 0, [[STEP, nrow], [1, ELEM]])

        gpool = ctx.enter_context(tc.tile_pool(name="g", bufs=2))
        dpool = ctx.enter_context(tc.tile_pool(name="d", bufs=2))
        vpool = ctx.enter_context(tc.tile_pool(name="v", bufs=2))
        hpool = ctx.enter_context(tc.tile_pool(name="h", bufs=2))
        fpool = ctx.enter_context(tc.tile_pool(name="f", bufs=2))
        opool = ctx.enter_context(tc.tile_pool(name="o", bufs=3))

        V32_tiles = {}
        o_tiles = {}
        G_cur = None
        for m in [mm for _r in range(reps) for mm in range(NBOX)]:
            gg, go = m // GB, m % GB
            if go == 0:
                G_cur = gpool.tile([128, GB * C, ELEM], f16, tag="G")
                nc.gpsimd.dma_gather(
                    out_ap=G_cur[:], in_ap=in_view,
                    idxs_ap=gidx_s[:, (gg % NG) * GW:(gg % NG + 1) * GW],
                    num_idxs=NGIDX, num_idxs_reg=NGIDX,
                    elem_size=ELEM, elem_step=STEP,
                )
            ag, ao = m // AB, m % AB
            if ao == 0:
                V32_tiles[ag] = vpool.tile([128, AB, C, XW], f32, tag="V",
                                           name=f"V{ag}")

            # vertical blend: V32 = R0 + wy*(R1-R0)
            Gm = G_cur[:].rearrange("p (b c) e -> p b c e", b=GB)
            R0 = Gm[:, go, :, 0:XW]
            R1 = Gm[:, go, :, W:W + XW]
            Dv = dpool.tile([128, C, XW], f16, tag="Dv")
            nc.vector.tensor_tensor(out=Dv[:], in0=R1, in1=R0,
                                    op=Alu.subtract)
            nc.vector.scalar_tensor_tensor(
                out=V32_tiles[ag][:, ao], in0=Dv[:],
                scalar=wyT_s[:, (m % NBOX):(m % NBOX) + 1], in1=R0,
                op0=Alu.mult, op1=Alu.add)

            if ao == AB - 1:
                V32 = V32_tiles.pop(ag)
                Hh = hpool.tile([128, AB, C, 2, S], f32, tag="Hh")
                nc.gpsimd.ap_gather(
                    out_ap=Hh[:].rearrange("p b c u j -> p (b c u j) ()"),
                    in_ap=V32[:].rearrange("p b c e -> p (b c e) ()"),
                    idxs_ap=agidx_s[:, (ag % NA) * AW:(ag % NA + 1) * AW],
                    channels=128, num_elems=AB * C * XW, d=1,
                    num_idxs=NAIDX,
                )
                o4 = opool.tile([128, AB, C, S], f16, tag="o4")
                o_tiles[ag] = o4
                for b2 in range(AB):
                    m2 = (ag % NA) * AB + b2
                    H0 = Hh[:, b2, :, 0, :]
                    H1 = Hh[:, b2, :, 1, :]
                    Dh = fpool.tile([128, C, S], f32, tag="Dh")
                    nc.vector.tensor_tensor(out=Dh[:], in0=H1, in1=H0,
                                            op=Alu.subtract)
                    wall_ap = W_all[:]
                    Wap = bass.AP(
                        wall_ap.tensor, wall_ap.offset + m2 * S,
                        [list(wall_ap.ap[0]), [0, C], [1, S]])
                    Mh = fpool.tile([128, C, S], f32, tag="Mh")
                    nc.vector.tensor_tensor(out=Mh[:], in0=Dh[:], in1=Wap,
                                            op=Alu.mult)
                    nc.vector.tensor_tensor(out=o4[:, b2], in0=Mh[:], in1=H0,
                                            op=Alu.add)
                dst = out_d.ap()[(ag % NA) * AB:(ag % NA) * AB + AB]
                dst = dst.transpose([2, 0, 1, 3])  # [S(i), AB, C, S(j)]
                nc.sync.dma_start(dst, o_tiles.pop(ag)[:S])

    nc.compile()
    return nc


def _get_rt():
    if "rt" in _CACHE:
        return _CACHE["rt"]
    import jax
    from concourse import mybir
    from concourse.bass2jax import (_bass_exec_p, install_neuronx_cc_hook,
                                    partition_id_tensor)

    nc = _build_program()
    install_neuronx_cc_hook()
    partition_name = (nc.partition_id_tensor.name
                      if nc.partition_id_tensor else None)
    in_names, out_names, out_avals = [], [], []
    for alloc in nc.m.functions[0].allocations:
        if not isinstance(alloc, mybir.MemoryLocationSet):
            continue
        name = alloc.memorylocations[0].name
        if alloc.kind == "ExternalInput":
            if name != partition_name:
                in_names.append(name)
        elif alloc.kind == "ExternalOutput":
            out_names.append(name)
            out_avals.append(jax.core.ShapedArray(
                tuple(alloc.tensor_shape), mybir.dt.np(alloc.dtype)))
    n_params = len(in_names)
    all_names = (in_names + out_names
                 + ([partition_name] if partition_name else []))

    def _body(*args):
        operands = list(args)
        if partition_name is not None:
            operands.append(partition_id_tensor())
        outs = _bass_exec_p.bind(
            *operands, out_avals=tuple(out_avals), in_names=tuple(all_names),
            out_names=tuple(out_names), lowering_input_output_aliases=(),
            sim_require_finite=True, sim_require_nnan=True, nc=nc)
        return tuple(outs)

    jitfn = jax.jit(_body, donate_argnums=(n_params,), keep_unused=True)
    from concurrent.futures import ThreadPoolExecutor
    rt = {
        "jax": jax, "nc": nc, "jitfn": jitfn, "in_names": in_names,
        "out_shape": tuple(out_avals[0].shape),
        "out_dtype": out_avals[0].dtype,
        "devices": jax.devices()[:B], "last_out": [None] * B,
        "pool": ThreadPoolExecutor(max_workers=2),
    }
    _CACHE["rt"] = rt
    return rt


def kernel(images: np.ndarray, boxes: np.ndarray) -> np.ndarray:
    images = np.asarray(images, dtype=np.float32)
    boxes = np.asarray(boxes, dtype=np.float32)
    assert images.shape == (B, C, H, W) and boxes.shape == (B, NBOX, 4)

    rt = _get_rt()
    jax = rt["jax"]
    import jax.numpy as jnp

    futs = [rt["pool"].submit(_prep_core, images[b], boxes[b])
            for b in range(B)]
    outs = []
    for b in range(B):
        m = futs[b].result()
        darr = [jax.device_put(m[n], rt["devices"][b]) for n in rt["in_names"]]
        z = rt["last_out"][b]
        if z is None:
            z = jax.device_put(
                jnp.zeros(rt["out_shape"], rt["out_dtype"]), rt["devices"][b])
        o = rt["jitfn"](*darr, z)[0]
        rt["last_out"][b] = o
        o.copy_to_host_async()
        outs.append(o)

    out = np.empty((B * NBOX, C, S, S), np.float32)
    for b in range(B):
        out[b * NBOX:(b + 1) * NBOX] = np.asarray(outs[b])
    return out


if __name__ == "__main__":
    import reference
    inputs = {k: np.asarray(v) for k, v in reference.setup_inputs().items()}
    got = kernel(**inputs)
    expected = np.asarray(reference.reference(**inputs))
    err = np.linalg.norm((got - expected).ravel()) / np.linalg.norm(
        np.asarray(expected).ravel())
    print("rel l2:", err)
